# revision 6
# baseline (speedup 1.0000x reference)
"""FOFEReader Trainium2 kernel: 8-core SPMD (batch x s-half sharding).

Math (per batch b, candidate (s, e=s+j), j<16):
  F[t] = sum_{k<=t} a^(t-k) doc[k]      (prefix FOFE),  R[t] = sum_{k>=t} a^(k-t) doc[k]
  x = [F[s-1] | F[s+j] - a^(j+1) F[s-1] | R[s+j+1] | qf]
  out = (relu(bn2(relu(bn1(x @ W1.T)) @ W2.T)) @ W3.T)
Reformulated so the 1212-dim GEMM is shared across the 16 spans j:
  G_u = U_u @ F (u in {l,c}), G_r = U_r @ R   with W1.T = [U_l U_c U_r U_q] row blocks
  z1[s,j] = (G_l[s-1] + q1) + (G_c[s+j] + G_r[s+j+1]) - a^(j+1) G_c[s-1]
All matmuls run in fp32r (TF32) at full PE rate; batchnorm is applied as
per-partition scale/bias in the ScalarE eviction (no weight preprocessing).
Each core handles one batch and one half of the s range (405 starts + halo).
"""
import os
import sys

for _p in ("/opt/trn_rl_repo", "/root/.axon_site/_ro/trn_rl_repo"):
    if os.path.isdir(_p) and _p not in sys.path:
        sys.path.insert(0, _p)
        break

import numpy as np

T = 809
MSPAN = 16
B = 4
ALPHA = 0.9
NS = 406          # s-starts per core (even: f32r matmul needs even free dim)
WIN = 424         # t window per core: t = s_lo-1 + i, i in [0, 424)
DD = 304
EMB = 300
LQ = 30
H4 = 1024
H2 = 512
BN_EPS = 1e-5
N_CORES = 8

_CACHE = {}


def _round_tf32(a):
    a = np.ascontiguousarray(a, dtype=np.float32)
    return (a.view(np.uint32) & np.uint32(0xFFFFE000)).view(np.float32)


def _build_amat(s_lo):
    """[809, 2*WIN] fp32: cols 0..WIN-1 = forward-FOFE operator columns for
    t=s_lo-1+i (A^T slice), cols WIN.. = reverse. Out-of-range t -> zero col."""
    t_idx = s_lo - 1 + np.arange(WIN)
    kv = np.arange(T)[:, None]
    tv = t_idx[None, :]
    valid = ((t_idx >= 0) & (t_idx <= T - 1))[None, :]
    af = np.where((kv <= tv) & valid, ALPHA ** np.maximum(tv - kv, 0), 0.0)
    ar = np.where((kv >= tv) & valid, ALPHA ** np.maximum(kv - tv, 0), 0.0)
    return _round_tf32(np.concatenate([af, ar], axis=1))


def _cand_indices():
    s_list, e_list = [], []
    for s in range(T):
        for span in range(min(MSPAN, T - s)):
            s_list.append(s)
            e_list.append(s + span)
    return np.asarray(s_list, np.int64), np.asarray(e_list, np.int64)


def _build_bass():
    import concourse.bacc as bacc
    import concourse.tile as tile
    from concourse import mybir
    from contextlib import ExitStack

    F32 = mybir.dt.float32
    F32R = mybir.dt.float32r
    AF = mybir.ActivationFunctionType
    OP = mybir.AluOpType

    nc = bacc.Bacc("TRN2", target_bir_lowering=False, debug=False,
                   num_devices=N_CORES)

    doc = nc.dram_tensor("doc", [T, DD], F32R, kind="ExternalInput").ap()
    amat = nc.dram_tensor("amat", [T, 2 * WIN], F32R, kind="ExternalInput").ap()
    query = nc.dram_tensor("query", [LQ, EMB], F32, kind="ExternalInput").ap()
    wvec = nc.dram_tensor("wvec", [LQ, 1], F32, kind="ExternalInput").ap()
    w1t = nc.dram_tensor("w1t", [3 * DD + EMB, H4], F32R, kind="ExternalInput").ap()
    w2t = nc.dram_tensor("w2t", [H4, H2], F32R, kind="ExternalInput").ap()
    w3t = nc.dram_tensor("w3t", [H2, 2], F32R, kind="ExternalInput").ap()
    bn1 = nc.dram_tensor("bn1", [128, 4, 8], F32, kind="ExternalInput").ap()
    bn2 = nc.dram_tensor("bn2", [128, 4, 4], F32, kind="ExternalInput").ap()
    y = nc.dram_tensor("y", [MSPAN, 2, NS], F32, kind="ExternalOutput").ap()

    KT1 = [(0, 128), (128, 128), (256, 48)]       # d-tiles of 304
    KTQ = [(0, 128), (128, 128), (256, 44)]       # e-tiles of 300
    KDOC = [(k, min(128, T - k)) for k in range(0, T, 128)]   # 7 k-tiles of 809
    NHT = H4 // 128   # 8
    NMT = H2 // 128   # 4

    with ExitStack() as ctx:
        tc = ctx.enter_context(tile.TileContext(nc))
        const = ctx.enter_context(tc.tile_pool(name="const", bufs=1))
        stream = ctx.enter_context(tc.tile_pool(name="stream", bufs=2))
        work = ctx.enter_context(tc.tile_pool(name="work", bufs=4))
        h1p = ctx.enter_context(tc.tile_pool(name="h1p", bufs=12))
        h2p = ctx.enter_context(tc.tile_pool(name="h2p", bufs=8))
        outp = ctx.enter_context(tc.tile_pool(name="outp", bufs=3))
        ps = ctx.enter_context(tc.tile_pool(name="ps", bufs=8, space="PSUM"))

        # ---- weights to SBUF ----
        w1_sb = {}  # (u, kt) -> [ksz, 1024]
        for u in range(4):
            base_row = u * DD
            kts = KT1 if u < 3 else KTQ
            for kt, (k0, ksz) in enumerate(kts):
                t_ = const.tile([128, H4], F32R, tag=f"w1_{u}_{kt}")
                nc.sync.dma_start(out=t_[:ksz], in_=w1t[base_row + k0: base_row + k0 + ksz, :])
                w1_sb[(u, kt)] = t_
        w2_sb = []
        for kt in range(NHT):
            t_ = const.tile([128, H2], F32R, tag=f"w2_{kt}")
            nc.sync.dma_start(out=t_, in_=w2t[kt * 128:(kt + 1) * 128, :])
            w2_sb.append(t_)
        w3_sb = []
        for mt in range(NMT):
            t_ = const.tile([128, 2], F32R, tag=f"w3_{mt}")
            nc.sync.dma_start(out=t_, in_=w3t[mt * 128:(mt + 1) * 128, :])
            w3_sb.append(t_)

        # ---- batchnorm scale/shift ----
        bn1_sb = const.tile([128, 4, 8], F32, tag="bn1")
        bn2_sb = const.tile([128, 4, 4], F32, tag="bn2")
        nc.sync.dma_start(out=bn1_sb, in_=bn1)
        nc.sync.dma_start(out=bn2_sb, in_=bn2)

        eps_sb = const.tile([128, 1], F32, tag="eps")
        nc.vector.memset(eps_sb, BN_EPS)

        def bn_prep(src, n):
            g, b_, m, v = (src[:, i, :] for i in range(4))
            sd = const.tile([128, n], F32, tag=f"sd{n}")
            nc.scalar.activation(out=sd, in_=v, func=AF.Sqrt, bias=eps_sb, scale=1.0)
            rs = const.tile([128, n], F32, tag=f"rs{n}")
            nc.vector.reciprocal(out=rs, in_=sd)
            sc = const.tile([128, n], F32, tag=f"sc{n}")
            nc.vector.tensor_mul(sc, g, rs)
            tmp = const.tile([128, n], F32, tag=f"tmp{n}")
            nc.vector.tensor_mul(tmp, m, sc)
            sh = const.tile([128, n], F32, tag=f"sh{n}")
            nc.vector.tensor_sub(sh, b_, tmp)
            return sc, sh

        scale1, shift1 = bn_prep(bn1_sb, NHT)
        scale2, shift2 = bn_prep(bn2_sb, NMT)

        # ---- query FOFE: qf = query.T @ wvec, q1 = U_q.T @ qf ----
        q_sb = const.tile([LQ, EMB], F32, tag="q_sb")
        nc.sync.dma_start(out=q_sb, in_=query)
        wv_sb = const.tile([LQ, 1], F32, tag="wv_sb")
        nc.sync.dma_start(out=wv_sb, in_=wvec)
        ps_qf = ps.tile([128, 3], F32, tag="ps")
        for kt, (k0, ksz) in enumerate(KTQ):
            nc.tensor.matmul(ps_qf[:ksz, kt:kt + 1], q_sb[:, k0:k0 + ksz], wv_sb[:],
                             start=True, stop=True)
        qf_sb = const.tile([128, 3], F32, tag="qf_sb")
        nc.scalar.activation(out=qf_sb, in_=ps_qf, func=AF.Copy)
        q1_sb = const.tile([128, NHT], F32, tag="q1_sb")
        for ht in range(NHT):
            ps_q1 = ps.tile([128, 1], F32, tag="ps")
            for kt, (k0, ksz) in enumerate(KTQ):
                nc.tensor.matmul(ps_q1,
                                 w1_sb[(3, kt)][:ksz, ht * 128:(ht + 1) * 128].bitcast(F32),
                                 qf_sb[:ksz, kt:kt + 1],
                                 start=(kt == 0), stop=(kt == 2))
            nc.scalar.activation(out=q1_sb[:, ht:ht + 1], in_=ps_q1, func=AF.Copy)

        # ---- F/R prefix GEMMs: [304, WIN] each, streamed over doc k-tiles ----
        ps_f = [ps.tile([128, WIN], F32, tag="ps", name=f"ps_f{i}") for i in range(3)]
        ps_r = [ps.tile([128, WIN], F32, tag="ps", name=f"ps_r{i}") for i in range(3)]
        nk = len(KDOC)
        for kt, (k0, ksz) in enumerate(KDOC):
            a_t = stream.tile([128, 2 * WIN], F32R, tag="amat")
            nc.sync.dma_start(out=a_t[:ksz], in_=amat[k0:k0 + ksz, :])
            d_t = stream.tile([128, DD], F32R, tag="doc")
            nc.sync.dma_start(out=d_t[:ksz], in_=doc[k0:k0 + ksz, :])
            for dt, (d0, dsz) in enumerate(KT1):
                nc.tensor.matmul(ps_f[dt][:dsz], d_t[:ksz, d0:d0 + dsz],
                                 a_t[:ksz, 0:WIN], start=(kt == 0), stop=(kt == nk - 1))
                nc.tensor.matmul(ps_r[dt][:dsz], d_t[:ksz, d0:d0 + dsz],
                                 a_t[:ksz, WIN:2 * WIN], start=(kt == 0), stop=(kt == nk - 1))
        f_sb, r_sb = [], []
        for dt, (d0, dsz) in enumerate(KT1):
            ft = const.tile([128, WIN], F32R, tag=f"f{dt}")
            nc.scalar.activation(out=ft[:dsz], in_=ps_f[dt][:dsz], func=AF.Copy)
            f_sb.append(ft)
            rt = const.tile([128, WIN], F32R, tag=f"r{dt}")
            nc.scalar.activation(out=rt[:dsz], in_=ps_r[dt][:dsz], func=AF.Copy)
            r_sb.append(rt)

        # ---- G GEMMs + eviction into Gc / base / E ----
        gc_sb, base_sb, e_sb = [], [], []
        for ht in range(NHT):
            hs = slice(ht * 128, (ht + 1) * 128)
            ps_gc = ps.tile([128, WIN], F32, tag="ps")
            for kt, (k0, ksz) in enumerate(KT1):
                nc.tensor.matmul(ps_gc, w1_sb[(1, kt)][:ksz, hs], f_sb[kt][:ksz],
                                 start=(kt == 0), stop=(kt == 2))
            gc = const.tile([128, WIN], F32, tag=f"gc{ht}")
            nc.scalar.activation(out=gc, in_=ps_gc, func=AF.Copy)
            gc_sb.append(gc)

            ps_gl = ps.tile([128, WIN], F32, tag="ps")
            for kt, (k0, ksz) in enumerate(KT1):
                nc.tensor.matmul(ps_gl, w1_sb[(0, kt)][:ksz, hs], f_sb[kt][:ksz],
                                 start=(kt == 0), stop=(kt == 2))
            bs = const.tile([128, NS], F32, tag=f"base{ht}")
            nc.scalar.activation(out=bs, in_=ps_gl[:, 0:NS], func=AF.Identity,
                                 bias=q1_sb[:, ht:ht + 1], scale=1.0)
            base_sb.append(bs)

            ps_gr = ps.tile([128, WIN], F32, tag="ps")
            for kt, (k0, ksz) in enumerate(KT1):
                nc.tensor.matmul(ps_gr, w1_sb[(2, kt)][:ksz, hs], r_sb[kt][:ksz],
                                 start=(kt == 0), stop=(kt == 2))
            et = const.tile([128, WIN], F32, tag=f"e{ht}")
            nc.vector.tensor_tensor(out=et[:, 1:423], in0=ps_gr[:, 2:424],
                                    in1=gc[:, 1:423], op=OP.add)
            e_sb.append(et)

        # ---- main loop over spans j ----
        for j in range(MSPAN):
            cjv = float(-(ALPHA ** (j + 1)))
            h1_tiles = []
            for ht in range(NHT):
                s1 = work.tile([128, NS], F32, tag="s1")
                nc.vector.scalar_tensor_tensor(out=s1, in0=gc_sb[ht][:, 0:NS],
                                               scalar=cjv, in1=base_sb[ht],
                                               op0=OP.mult, op1=OP.add)
                z = work.tile([128, NS], F32, tag="z")
                nc.gpsimd.tensor_tensor(out=z, in0=s1,
                                        in1=e_sb[ht][:, j + 1:j + 1 + NS], op=OP.add)
                h1 = h1p.tile([128, NS], F32R, tag="h1")
                nc.scalar.activation(out=h1, in_=z, func=AF.Relu,
                                     bias=shift1[:, ht:ht + 1],
                                     scale=scale1[:, ht:ht + 1])
                h1_tiles.append(h1)
            h2_tiles = []
            for mt in range(NMT):
                ps_l2 = ps.tile([128, NS], F32, tag="ps")
                ms = slice(mt * 128, (mt + 1) * 128)
                for kt in range(NHT):
                    nc.tensor.matmul(ps_l2, w2_sb[kt][:, ms], h1_tiles[kt][:],
                                     start=(kt == 0), stop=(kt == NHT - 1))
                h2 = h2p.tile([128, NS], F32R, tag="h2")
                nc.scalar.activation(out=h2, in_=ps_l2, func=AF.Relu,
                                     bias=shift2[:, mt:mt + 1],
                                     scale=scale2[:, mt:mt + 1])
                h2_tiles.append(h2)
            ps_l3 = ps.tile([2, NS], F32, tag="ps")
            for mt in range(NMT):
                nc.tensor.matmul(ps_l3, w3_sb[mt][:], h2_tiles[mt][:],
                                 start=(mt == 0), stop=(mt == NMT - 1))
            o = outp.tile([2, NS], F32, tag="o")
            nc.scalar.activation(out=o, in_=ps_l3, func=AF.Copy)
            nc.sync.dma_start(out=y[j], in_=o)

    nc.compile()
    return nc


def _get_nc():
    if "nc" not in _CACHE:
        _CACHE["nc"] = _build_bass()
    return _CACHE["nc"]


def _ensure_device():
    """Probe the axon device; reset it if wedged."""
    if _CACHE.get("dev_ok"):
        return
    import jax
    import jax.numpy as jnp
    try:
        (jnp.zeros((8, 8)) + 1).block_until_ready()
    except Exception:
        import ctypes
        lib = ctypes.CDLL("/opt/axon/libaxon_pjrt.so")
        lib.axon_reset.restype = ctypes.c_int64
        jax.devices()
        lib.axon_reset()
        (jnp.zeros((8, 8)) + 1).block_until_ready()
    _CACHE["dev_ok"] = True


def _make_in_maps(inputs):
    doc_emb = np.asarray(inputs["doc_emb"], np.float32)
    query_emb = np.asarray(inputs["query_emb"], np.float32)
    w1tt = _round_tf32(np.asarray(inputs["W1"], np.float32).T)
    w2tt = _round_tf32(np.asarray(inputs["W2"], np.float32).T)
    w3tt = _round_tf32(np.asarray(inputs["W3"], np.float32).T)
    wv = _round_tf32(ALPHA ** np.arange(LQ - 1, -1, -1, dtype=np.float32))[:, None]
    bn1 = np.stack([np.asarray(inputs[k], np.float32).reshape(NHT8, 128).T
                    for k in ("g1", "b1", "m1", "v1")], axis=1)
    bn2 = np.stack([np.asarray(inputs[k], np.float32).reshape(4, 128).T
                    for k in ("g2", "b2", "m2", "v2")], axis=1)
    amats = [_build_amat(0), _build_amat(403)]
    in_maps = []
    for core in range(N_CORES):
        b, half = core // 2, core % 2
        in_maps.append({
            "doc": _round_tf32(doc_emb[b]),
            "amat": amats[half],
            "query": _round_tf32(query_emb[b]),
            "wvec": wv,
            "w1t": w1tt,
            "w2t": w2tt,
            "w3t": w3tt,
            "bn1": np.ascontiguousarray(bn1),
            "bn2": np.ascontiguousarray(bn2),
        })
    return in_maps


NHT8 = H4 // 128


def _gather(results):
    s_idx, e_idx = _CACHE.setdefault("cands", _cand_indices())
    n = len(s_idx)
    j_idx = e_idx - s_idx
    half_idx = (s_idx >= 406).astype(np.int64)
    u_idx = s_idx - 403 * half_idx
    out = np.zeros((B, n, 2), np.float32)
    for b in range(B):
        both = np.stack([results[2 * b]["y"], results[2 * b + 1]["y"]])  # [2,16,2,NS]
        out[b] = both[half_idx, j_idx, :, u_idx]
    return out


def _run(inputs, trace=False):
    from concourse import bass_utils
    _ensure_device()
    nc = _get_nc()
    in_maps = _make_in_maps(inputs)
    res = bass_utils.run_bass_kernel_spmd(nc, in_maps,
                                          core_ids=list(range(N_CORES)),
                                          trace=trace)
    return _gather(res.results), res


def kernel(**inputs) -> np.ndarray:
    out, _ = _run(inputs, trace=False)
    return out
